# revision 5
# baseline (speedup 1.0000x reference)
"""GPT forward (L=12, D=1024, H=16, B=2, T=1024, V=32000) on 8 trn2 NeuronCores.

Sharding: sequence-parallel. Core c owns batch c//4, token chunk c%4 (256 tokens).
Per layer: LN (token-major, fp32) -> PE-transpose to feature-major bf16 ->
K matmuls -> ship K^T -> AllGather-K (per-batch 4-core groups) overlapped with
Q and V matmuls -> ship V -> AllGather-V -> masked attention in transposed-score
layout, software-pipelined: scores packed in [128,1024] PSUM quads, exp on
scalar, mask-mul split vector/pool, AV matmuls delayed one head so the PE never
waits on the softmax chain (denominator via ones-column in the V matmul) ->
proj (+residual) -> LN2 -> fc1 -> exact GELU -> fc2 (+residual).
Next-layer weight DMAs are emitted mid-layer so transfers hide under compute.
Final: LN -> AllGather of hidden states (all 8 cores, Shared-output) ->
vocab-sharded logits matmul (each core: 2048 tokens x its 4000 vocab columns).
Everything bf16 into the PE with fp32 PSUM accumulation; residual stream fp32.
"""
import sys
import numpy as np

sys.path.insert(0, '/opt/trn_rl_repo')
import ml_dtypes

BF = ml_dtypes.bfloat16
L, D, H, V, B, T = 12, 1024, 16, 32000, 2, 1024
DH = D // H          # 64
EPS = 1e-5
N_CORES = 8
CHUNK = 256          # tokens per core
VS = V // N_CORES    # 4000 vocab cols per core
KT = 8               # kv tiles of 128 per batch
QT = KT // 4         # score quads ([128, 1024] = 4 kv tiles) per head
FT = D // 128        # 8 feature tiles


def host_prep(inputs):
    inputs = {k: np.asarray(v) for k, v in inputs.items()}
    for name in ['ln1_b', 'ln2_b', 'b_qkv', 'b_proj', 'b_fc1', 'b_fc2', 'lnf_b']:
        assert not np.any(inputs[name]), f"{name} nonzero — bias folding unsupported"
    x0 = inputs['wte'][inputs['tokens']] + inputs['wpe'][None, :, :]   # [B,T,D] f32
    w_qkv = inputs['w_qkv'] * inputs['ln1_w'][:, :, None]
    w_fc1 = inputs['w_fc1'] * inputs['ln2_w'][:, :, None]
    w_out = inputs['w_out'] * inputs['lnf_w'][:, None]
    return {
        'x0': np.ascontiguousarray(x0, np.float32),
        'w_qkv': np.ascontiguousarray(w_qkv.astype(BF)),
        'w_proj': np.ascontiguousarray(inputs['w_proj'].astype(BF)),
        'w_fc1': np.ascontiguousarray(w_fc1.astype(BF)),
        'w_fc2': np.ascontiguousarray(inputs['w_fc2'].astype(BF)),
        'w_out': np.ascontiguousarray(w_out.astype(BF)),
    }


def make_masks(j):
    """Causal masks, transposed-score layout, packed in quads of 4 kv tiles:
    mask[p][i, 256*t' + q] = (128*(4p+t') + i) <= (256j + q)."""
    out = np.zeros((QT, 128, 4 * CHUNK), BF)
    for p in range(QT):
        for tp in range(4):
            t = 4 * p + tp
            kv = 128 * t + np.arange(128)[:, None]
            qp = 256 * j + np.arange(CHUNK)[None, :]
            out[p][:, 256 * tp:256 * (tp + 1)] = (kv <= qp).astype(BF)
    return out


def build_program(n_layers=L, bcast_mode='dve0'):
    import concourse.bass as bass
    import concourse.mybir as mybir
    import concourse.tile as tile
    from concourse import bacc
    from concourse.masks import make_identity
    from contextlib import ExitStack

    f32 = mybir.dt.float32
    bf16 = mybir.dt.bfloat16
    AF = mybir.ActivationFunctionType

    nc = bacc.Bacc('TRN2', target_bir_lowering=False, debug=False, num_devices=N_CORES)

    x0_in = nc.dram_tensor("x0", [CHUNK, D], f32, kind="ExternalInput")
    wq_in = nc.dram_tensor("wq", [n_layers, D, 3 * D], bf16, kind="ExternalInput")
    wp_in = nc.dram_tensor("wp", [n_layers, D, D], bf16, kind="ExternalInput")
    w1_in = nc.dram_tensor("w1", [n_layers, D, D], bf16, kind="ExternalInput")
    w2_in = nc.dram_tensor("w2", [n_layers, D, D], bf16, kind="ExternalInput")
    wo_in = nc.dram_tensor("wo", [D, VS], bf16, kind="ExternalInput")
    mk_in = nc.dram_tensor("masks", [QT, 128, 4 * CHUNK], bf16, kind="ExternalInput")
    out_ext = nc.dram_tensor("logits", [N_CORES * CHUNK, VS], f32, kind="ExternalOutput")

    # per-layer collective buffers
    k_locs = [nc.dram_tensor(f"kl_{l}", [D, CHUNK], bf16) for l in range(n_layers)]
    k_alls = [nc.dram_tensor(f"ka_{l}", [4 * D, CHUNK], bf16) for l in range(n_layers)]
    v_locs = [nc.dram_tensor(f"vl_{l}", [CHUNK, D], bf16) for l in range(n_layers)]
    v_alls = [nc.dram_tensor(f"va_{l}", [4 * CHUNK, D], bf16) for l in range(n_layers)]
    xf_loc = nc.dram_tensor("xfl", [D, CHUNK], bf16)
    xf_all = nc.dram_tensor("xfa", [N_CORES * D, CHUNK], bf16, addr_space="Shared")

    groups_b = [[0, 1, 2, 3], [4, 5, 6, 7]]
    group_all = [list(range(N_CORES))]

    def dram_ap(handle, offset, ap):
        base = handle[:, :]
        return bass.AP(tensor=base.tensor, offset=offset, ap=ap)

    def _patch_tile_name(pool):
        orig = pool.tile
        def tile(shape, dtype, *, tag="", **kw):
            kw.setdefault("name", tag or "t")
            return orig(shape, dtype, tag=tag, **kw)
        pool.tile = tile
        return pool

    with tile.TileContext(nc) as tc, ExitStack() as ctx:
        persist = _patch_tile_name(ctx.enter_context(tc.tile_pool(name="persist", bufs=1)))
        x_t = [persist.tile([128, D], f32, tag=f"x{m}") for m in range(2)]
        for m in range(2):
            nc.sync.dma_start(out=x_t[m], in_=x0_in[128 * m:128 * (m + 1), :])
        ident = persist.tile([128, 128], bf16, tag="ident")
        make_identity(nc, ident)
        eps_t = persist.tile([128, 1], f32, tag="eps")
        nc.vector.memset(eps_t, EPS)
        mask_t = [persist.tile([128, 4 * CHUNK], bf16, tag=f"mask{p}") for p in range(QT)]
        for p in range(QT):
            nc.sync.dma_start(out=mask_t[p], in_=mk_in[p, :, :])
        ones1 = persist.tile([1, 64], bf16, tag="ones1")
        nc.vector.memset(ones1, 1.0)

        ln_pool = _patch_tile_name(ctx.enter_context(tc.tile_pool(name="ln", bufs=1)))
        wqpool = _patch_tile_name(ctx.enter_context(tc.tile_pool(name="wq", bufs=1)))
        wpool = _patch_tile_name(ctx.enter_context(tc.tile_pool(name="wsmall", bufs=1)))
        apool = _patch_tile_name(ctx.enter_context(tc.tile_pool(name="acts", bufs=1)))
        kvpool = _patch_tile_name(ctx.enter_context(tc.tile_pool(name="kv", bufs=1)))
        epool = _patch_tile_name(ctx.enter_context(tc.tile_pool(name="eexp", bufs=1)))
        spool = _patch_tile_name(ctx.enter_context(tc.tile_pool(name="small", bufs=3)))

        ps_q = _patch_tile_name(ctx.enter_context(tc.tile_pool(name="ps_q", bufs=2, space="PSUM")))
        ps_g = _patch_tile_name(ctx.enter_context(tc.tile_pool(name="ps_g", bufs=4, space="PSUM")))

        def layernorm_T(xt, tagpfx):
            """LN both token tiles of x -> feature-major bf16 tiles [128, 256] x8."""
            xh = []
            for m in range(2):
                stats = spool.tile([128, 2, 6], f32, tag="lnstats")
                nc.vector.bn_stats(out=stats[:, 0, :], in_=xt[m][:, 0:512])
                nc.vector.bn_stats(out=stats[:, 1, :], in_=xt[m][:, 512:1024])
                mv = spool.tile([128, 2], f32, tag="lnmv")
                nc.vector.bn_aggr(out=mv, in_=stats)
                rs = spool.tile([128, 1], f32, tag="lnrs")
                nc.scalar.activation(out=rs, in_=mv[:, 1:2], func=AF.Sqrt, bias=eps_t)
                nc.vector.reciprocal(out=rs, in_=rs)
                xh_m = ln_pool.tile([128, D], bf16, tag=f"lnxh{m}")
                nc.vector.tensor_scalar(
                    out=xh_m, in0=xt[m], scalar1=mv[:, 0:1], scalar2=rs,
                    op0=mybir.AluOpType.subtract, op1=mybir.AluOpType.mult)
                xh.append(xh_m)
            xhT = [ln_pool.tile([128, CHUNK], bf16, tag=f"lnxhT{t}") for t in range(FT)]
            for t in range(FT):
                for m in range(2):
                    ptr = ps_g.tile([128, 128], bf16, tag="g", padded_shape=[128, 512])
                    nc.tensor.transpose(ptr, xh[m][:, 128 * t:128 * (t + 1)], ident)
                    eng = nc.vector if (t + m) % 2 == 0 else nc.scalar
                    if eng is nc.scalar:
                        nc.scalar.copy(out=xhT[t][:, 128 * m:128 * (m + 1)], in_=ptr)
                    else:
                        nc.vector.tensor_copy(out=xhT[t][:, 128 * m:128 * (m + 1)], in_=ptr)
            return xhT

        def load_wq(li, what):
            """Emit weight DMAs for layer li's qkv weights. what in {'k','qv'}."""
            for kk in range(FT):
                r0, r1 = 128 * kk, 128 * (kk + 1)
                if what == 'k':
                    nc.sync.dma_start(out=wq_t[kk][:, D:2 * D], in_=wq_in[li, r0:r1, D:2 * D])
                else:
                    nc.sync.dma_start(out=wq_t[kk][:, 0:D], in_=wq_in[li, r0:r1, 0:D])
                    nc.sync.dma_start(out=wq_t[kk][:, 2 * D:3 * D], in_=wq_in[li, r0:r1, 2 * D:3 * D])

        def load_w(dst, src, li):
            for kk in range(FT):
                nc.sync.dma_start(out=dst[kk], in_=src[li, 128 * kk:128 * (kk + 1), :])

        # persistent weight tiles
        wq_t = [wqpool.tile([128, 3 * D], bf16, tag=f"wq{kk}") for kk in range(FT)]
        wp_t = [wpool.tile([128, D], bf16, tag=f"wp{kk}") for kk in range(FT)]
        w1_t = [wpool.tile([128, D], bf16, tag=f"w1{kk}") for kk in range(FT)]
        w2_t = [wpool.tile([128, D], bf16, tag=f"w2{kk}") for kk in range(FT)]

        # prologue: layer 0 weights
        load_wq(0, 'k')
        load_wq(0, 'qv')
        load_w(wp_t, wp_in, 0)
        load_w(w1_t, w1_in, 0)
        load_w(w2_t, w2_in, 0)

        def dense_chain_256(dst_tiles, f_range, col_of, xhT, copy_engines=None):
            """out feature-major tiles: for f in f_range: chain over kk, copy to dst."""
            for i, f in enumerate(f_range):
                ps = ps_g.tile([128, 512], f32, tag="g")
                for kk in range(FT):
                    nc.tensor.matmul(ps[:, 0:CHUNK], wq_t[kk][:, col_of(f):col_of(f) + 128],
                                     xhT[kk], start=(kk == 0), stop=(kk == FT - 1))
                eng = (copy_engines[i % len(copy_engines)] if copy_engines else nc.vector)
                if eng is nc.scalar:
                    nc.scalar.copy(out=dst_tiles[i], in_=ps[:, 0:CHUNK])
                else:
                    eng.tensor_copy(out=dst_tiles[i], in_=ps[:, 0:CHUNK])

        def forward():
            for l in range(n_layers):
                xhT = layernorm_T(x_t, f"ln1_{l}")

                # ---- K first, ship, AllGather-K ----
                kTl = [apool.tile([128, CHUNK], bf16, tag=f"kT{t}") for t in range(FT)]
                dense_chain_256(kTl, range(FT), lambda f: D + 128 * f, xhT,
                                [nc.vector, nc.scalar])
                k_loc, k_all = k_locs[l], k_alls[l]
                for t in range(FT):
                    nc.sync.dma_start(
                        out=dram_ap(k_loc, 128 * t * CHUNK, [[CHUNK, 128], [1, CHUNK]]),
                        in_=kTl[t])
                nc.gpsimd.collective_compute(
                    "AllGather", mybir.AluOpType.bypass, replica_groups=groups_b,
                    ins=[k_loc[:, :]], outs=[k_all[:, :]])

                # ---- Q (fills AG-K window) ----
                qT = [apool.tile([128, CHUNK], bf16, tag=f"qT{t}") for t in range(FT)]
                dense_chain_256(qT, range(FT), lambda f: 128 * f, xhT,
                                [nc.vector, nc.scalar])

                # ---- V, ship, AllGather-V ----
                v_t = [kvpool.tile([128, D], bf16, tag=f"vloc{m}") for m in range(2)]
                for m in range(2):
                    for n in range(2):
                        ps = ps_g.tile([128, 512], f32, tag="g")
                        for kk in range(FT):
                            nc.tensor.matmul(
                                ps, xhT[kk][:, 128 * m:128 * (m + 1)],
                                wq_t[kk][:, 2 * D + 512 * n:2 * D + 512 * (n + 1)],
                                start=(kk == 0), stop=(kk == FT - 1))
                        nc.vector.tensor_copy(out=v_t[m][:, 512 * n:512 * (n + 1)], in_=ps)
                v_loc, v_all = v_locs[l], v_alls[l]
                for m in range(2):
                    nc.sync.dma_start(
                        out=dram_ap(v_loc, 128 * m * D, [[D, 128], [1, D]]),
                        in_=v_t[m])
                nc.gpsimd.collective_compute(
                    "AllGather", mybir.AluOpType.bypass, replica_groups=groups_b,
                    ins=[v_loc[:, :]], outs=[v_all[:, :]])

                # prefetch next layer's qkv weights (transfers run during attention)
                if l + 1 < n_layers:
                    load_wq(l + 1, 'k')
                    load_wq(l + 1, 'qv')

                # ---- load gathered K^T: tiles [128, 1024] (features x kv tokens) ----
                kall = [kvpool.tile([128, 4 * CHUNK], bf16, tag=f"kall{t}") for t in range(FT)]
                for t in range(FT):
                    nc.sync.dma_start(
                        out=kall[t].rearrange("p (r c) -> p r c", r=4),
                        in_=dram_ap(k_all, 128 * t * CHUNK,
                                    [[CHUNK, 128], [D * CHUNK, 4], [1, CHUNK]]))
                # gathered V into 65-strided head-extended layout + ones col
                vext = [kvpool.tile([128, 16 * 65], bf16, tag=f"vext{t}") for t in range(KT)]
                for tt in range(KT):
                    r_, m_ = tt // 2, tt % 2
                    ve = vext[tt].rearrange("p (h c) -> p h c", h=16)
                    nc.sync.dma_start(
                        out=ve[:, :, 0:64],
                        in_=dram_ap(v_all, CHUNK * D * r_ + 128 * m_ * D,
                                    [[D, 128], [64, 16], [1, 64]]))
                    nc.gpsimd.memset(ve[:, :, 64:65], 1.0)

                # ---- attention: pipelined, av delayed one head ----
                attnT = [apool.tile([128, CHUNK], bf16, tag=f"kT{t}") for t in range(FT)]

                def emit_av(h, em_quads):
                    att_ps = ps_g.tile([65, CHUNK], f32, tag="g")
                    for t in range(KT):
                        nc.tensor.matmul(att_ps, vext[t][:, 65 * h:65 * h + 65],
                                         em_quads[t // 4][:, 256 * (t % 4):256 * (t % 4 + 1)],
                                         start=(t == 0), stop=(t == KT - 1))
                    ft, ro = h // 2, 64 * (h % 2)
                    r_sb = spool.tile([1, CHUNK], f32, tag="recip", bufs=3)
                    nc.vector.reciprocal(out=r_sb, in_=att_ps[64:65, :])
                    if bcast_mode == 'dve0':
                        rb = bass.AP(tensor=r_sb[:, :].tensor, offset=r_sb[:, :].offset,
                                     ap=[[0, 64]] + [list(p) for p in r_sb[:, :].ap[1:]])
                        nc.vector.tensor_mul(out=attnT[ft][ro:ro + 64, :],
                                             in0=att_ps[0:64, :], in1=rb)
                    elif bcast_mode == 'pb':
                        rb_sb = spool.tile([64, CHUNK], f32, tag="rbsb", bufs=2)
                        nc.gpsimd.partition_broadcast(rb_sb, r_sb, channels=64)
                        nc.vector.tensor_mul(out=attnT[ft][ro:ro + 64, :],
                                             in0=att_ps[0:64, :], in1=rb_sb)
                    else:  # 'mm'
                        r_bf = spool.tile([1, CHUNK], bf16, tag="recipb", bufs=3)
                        nc.vector.tensor_copy(out=r_bf, in_=r_sb)
                        rb_ps = ps_g.tile([64, CHUNK], f32, tag="g")
                        nc.tensor.matmul(rb_ps, ones1, r_bf, start=True, stop=True)
                        rb_sb = spool.tile([64, CHUNK], f32, tag="rbsb", bufs=2)
                        nc.vector.tensor_copy(out=rb_sb, in_=rb_ps)
                        nc.vector.tensor_mul(out=attnT[ft][ro:ro + 64, :],
                                             in0=att_ps[0:64, :], in1=rb_sb)

                prev = None
                for h in range(H):
                    ft, ro = h // 2, 64 * (h % 2)
                    em_quads = []
                    for p in range(QT):
                        s_ps = ps_q.tile([128, 4 * CHUNK], f32, tag="squad")
                        for tp in range(4):
                            t = 4 * p + tp
                            nc.tensor.matmul(
                                s_ps[:, 256 * tp:256 * (tp + 1)],
                                kall[ft][ro:ro + 64, 128 * t:128 * (t + 1)],
                                qT[ft][ro:ro + 64, :], start=True, stop=True)
                        e_q = epool.tile([128, 4 * CHUNK], bf16, tag="eq", bufs=3)
                        nc.scalar.activation(out=e_q, in_=s_ps, func=AF.Exp, scale=0.125)
                        em_q = epool.tile([128, 4 * CHUNK], bf16, tag="emq", bufs=4)
                        eng = nc.vector if p % 2 == 0 else nc.gpsimd
                        eng.tensor_mul(out=em_q, in0=e_q, in1=mask_t[p])
                        em_quads.append(em_q)
                    if prev is not None:
                        emit_av(*prev)
                    prev = (h, em_quads)
                emit_av(*prev)

                # ---- proj + residual ----
                for m in range(2):
                    for n in range(2):
                        ps = ps_g.tile([128, 512], f32, tag="g")
                        for kk in range(FT):
                            nc.tensor.matmul(
                                ps, attnT[kk][:, 128 * m:128 * (m + 1)],
                                wp_t[kk][:, 512 * n:512 * (n + 1)],
                                start=(kk == 0), stop=(kk == FT - 1))
                        nc.vector.tensor_add(
                            out=x_t[m][:, 512 * n:512 * (n + 1)],
                            in0=x_t[m][:, 512 * n:512 * (n + 1)], in1=ps)
                if l + 1 < n_layers:
                    load_w(wp_t, wp_in, l + 1)

                # ---- MLP ----
                hT = layernorm_T(x_t, f"ln2_{l}")
                gT = [apool.tile([128, CHUNK], bf16, tag=f"qT{t}") for t in range(FT)]
                for f in range(FT):
                    ps = ps_g.tile([128, 512], f32, tag="g")
                    for kk in range(FT):
                        nc.tensor.matmul(ps[:, 0:CHUNK], w1_t[kk][:, 128 * f:128 * (f + 1)],
                                         hT[kk], start=(kk == 0), stop=(kk == FT - 1))
                    nc.scalar.activation(out=gT[f], in_=ps[:, 0:CHUNK], func=AF.Gelu)
                if l + 1 < n_layers:
                    load_w(w1_t, w1_in, l + 1)
                for m in range(2):
                    for n in range(2):
                        ps = ps_g.tile([128, 512], f32, tag="g")
                        for kk in range(FT):
                            nc.tensor.matmul(
                                ps, gT[kk][:, 128 * m:128 * (m + 1)],
                                w2_t[kk][:, 512 * n:512 * (n + 1)],
                                start=(kk == 0), stop=(kk == FT - 1))
                        nc.vector.tensor_add(
                            out=x_t[m][:, 512 * n:512 * (n + 1)],
                            in0=x_t[m][:, 512 * n:512 * (n + 1)], in1=ps)
                if l + 1 < n_layers:
                    load_w(w2_t, w2_in, l + 1)

            # ---- final LN + AllGather (Shared) + logits ----
            xfT = layernorm_T(x_t, "lnf")
            for t in range(FT):
                nc.sync.dma_start(
                    out=dram_ap(xf_loc, 128 * t * CHUNK, [[CHUNK, 128], [1, CHUNK]]),
                    in_=xfT[t])
            nc.gpsimd.collective_compute(
                "AllGather", mybir.AluOpType.bypass, replica_groups=group_all,
                ins=[xf_loc[:, :]], outs=[xf_all[:, :]])
            xall = [wqpool.tile([128, N_CORES * CHUNK], bf16, tag=f"wq{t}")
                    for t in range(FT)]
            for t in range(FT):
                nc.sync.dma_start(
                    out=xall[t].rearrange("p (r c) -> p r c", r=N_CORES),
                    in_=dram_ap(xf_all, 128 * t * CHUNK,
                                [[CHUNK, 128], [D * CHUNK, N_CORES], [1, CHUNK]]))
            NCH = [512] * 7 + [VS - 512 * 7]
            for n in range(8):
                n0 = 512 * n
                won = [wpool.tile([128, 512], bf16, tag=f"won{kk}", bufs=2) for kk in range(FT)]
                for kk in range(FT):
                    nc.sync.dma_start(out=won[kk][:, :NCH[n]],
                                      in_=wo_in[128 * kk:128 * (kk + 1), n0:n0 + NCH[n]])
                for mm in range(16):
                    ps = ps_g.tile([128, 512], f32, tag="g")
                    for kk in range(FT):
                        nc.tensor.matmul(
                            ps[:, :NCH[n]], xall[kk][:, 128 * mm:128 * (mm + 1)],
                            won[kk][:, :NCH[n]],
                            start=(kk == 0), stop=(kk == FT - 1))
                    lg = ln_pool.tile([128, 512], f32, tag=f"lnxh{mm % 2}")
                    eng = nc.vector if mm % 2 == 0 else nc.scalar
                    if eng is nc.scalar:
                        nc.scalar.copy(out=lg[:, :NCH[n]], in_=ps[:, :NCH[n]])
                    else:
                        nc.vector.tensor_copy(out=lg[:, :NCH[n]], in_=ps[:, :NCH[n]])
                    nc.sync.dma_start(
                        out=out_ext[128 * mm:128 * (mm + 1), n0:n0 + NCH[n]],
                        in_=lg[:, :NCH[n]])

        forward()

    nc.compile()
    return nc


_CACHE = {}


def _get_program(n_layers=L, bcast_mode='dve0'):
    key = (n_layers, bcast_mode)
    if key not in _CACHE:
        _CACHE[key] = build_program(n_layers, bcast_mode)
    return _CACHE[key]


def build_in_maps(prep, n_layers=L):
    in_maps = []
    for c in range(N_CORES):
        b, j = c // 4, c % 4
        in_maps.append({
            'x0': np.ascontiguousarray(prep['x0'][b, 256 * j:256 * (j + 1), :]),
            'wq': prep['w_qkv'][:n_layers],
            'wp': prep['w_proj'][:n_layers],
            'w1': prep['w_fc1'][:n_layers],
            'w2': prep['w_fc2'][:n_layers],
            'wo': np.ascontiguousarray(prep['w_out'][:, VS * c:VS * (c + 1)]),
            'masks': make_masks(j),
        })
    return in_maps


def unshard(res, n_rep=1):
    parts = [res.results[c]['logits'] for c in range(N_CORES)]   # [2048, 4000] each
    full = np.concatenate(parts, axis=1)                          # [2048, 32000]
    return full.reshape(B, T, V)


def run_model(prep, n_layers=L, bcast_mode='dve0', **run_kwargs):
    from concourse.bass_utils import run_bass_kernel_spmd
    nc = _get_program(n_layers, bcast_mode)
    in_maps = build_in_maps(prep, n_layers)
    res = run_bass_kernel_spmd(nc, in_maps, core_ids=list(range(N_CORES)), **run_kwargs)
    return unshard(res)


def kernel(**inputs):
    prep = host_prep(inputs)
    return run_model(prep)


# revision 7
# speedup vs baseline: 1.5154x; 1.5154x over previous
"""GPT forward (L=12, D=1024, H=16, B=2, T=1024, V=32000) on 8 trn2 NeuronCores.

Sharding: sequence-parallel. Core c owns batch c//4, token chunk c%4 (256 tokens).
Per layer: LN (token-major, fp32) -> PE-transpose to feature-major bf16 ->
K matmuls -> ship K^T -> AllGather-K (per-batch 4-core groups) overlapped with
Q and V matmuls -> ship V -> AllGather-V -> masked attention in transposed-score
layout, software-pipelined: scores packed in [128,1024] PSUM quads, exp on
scalar, mask-mul split vector/pool, AV matmuls delayed one head so the PE never
waits on the softmax chain (denominator via ones-column in the V matmul) ->
proj (+residual) -> LN2 -> fc1 -> exact GELU -> fc2 (+residual).
Next-layer weight DMAs are emitted mid-layer so transfers hide under compute.
Final: LN -> AllGather of hidden states (all 8 cores, Shared-output) ->
vocab-sharded logits matmul (each core: 2048 tokens x its 4000 vocab columns).
Everything bf16 into the PE with fp32 PSUM accumulation; residual stream fp32.
"""
import sys
import numpy as np

sys.path.insert(0, '/opt/trn_rl_repo')
import ml_dtypes

BF = ml_dtypes.bfloat16
L, D, H, V, B, T = 12, 1024, 16, 32000, 2, 1024
DH = D // H          # 64
EPS = 1e-5
N_CORES = 8
CHUNK = 256          # tokens per core
VS = V // N_CORES    # 4000 vocab cols per core
KT = 8               # kv tiles of 128 per batch
QT = KT // 4         # score quads ([128, 1024] = 4 kv tiles) per head
FT = D // 128        # 8 feature tiles


def host_prep(inputs):
    inputs = {k: np.asarray(v) for k, v in inputs.items()}
    for name in ['ln1_b', 'ln2_b', 'b_qkv', 'b_proj', 'b_fc1', 'b_fc2', 'lnf_b']:
        assert not np.any(inputs[name]), f"{name} nonzero — bias folding unsupported"
    x0 = inputs['wte'][inputs['tokens']] + inputs['wpe'][None, :, :]   # [B,T,D] f32
    w_qkv = inputs['w_qkv'] * inputs['ln1_w'][:, :, None]
    w_fc1 = inputs['w_fc1'] * inputs['ln2_w'][:, :, None]
    w_out = inputs['w_out'] * inputs['lnf_w'][:, None]
    return {
        'x0': np.ascontiguousarray(x0, np.float32),
        'w_qkv': np.ascontiguousarray(w_qkv.astype(BF)),
        'w_proj': np.ascontiguousarray(inputs['w_proj'].astype(BF)),
        'w_fc1': np.ascontiguousarray(w_fc1.astype(BF)),
        'w_fc2': np.ascontiguousarray(inputs['w_fc2'].astype(BF)),
        'w_out': np.ascontiguousarray(w_out.astype(BF)),
    }


def make_masks(j):
    """Causal masks, transposed-score layout, packed in quads of 4 kv tiles:
    mask[p][i, 256*t' + q] = (128*(4p+t') + i) <= (256j + q)."""
    out = np.zeros((QT, 128, 4 * CHUNK), BF)
    for p in range(QT):
        for tp in range(4):
            t = 4 * p + tp
            kv = 128 * t + np.arange(128)[:, None]
            qp = 256 * j + np.arange(CHUNK)[None, :]
            out[p][:, 256 * tp:256 * (tp + 1)] = (kv <= qp).astype(BF)
    return out


def build_program(n_layers=L, bcast_mode='dve0'):
    import concourse.bass as bass
    import concourse.mybir as mybir
    import concourse.tile as tile
    from concourse import bacc
    from concourse.masks import make_identity
    from contextlib import ExitStack

    f32 = mybir.dt.float32
    bf16 = mybir.dt.bfloat16
    AF = mybir.ActivationFunctionType

    nc = bacc.Bacc('TRN2', target_bir_lowering=False, debug=False, num_devices=N_CORES)

    x0_in = nc.dram_tensor("x0", [CHUNK, D], f32, kind="ExternalInput")
    wq_in = nc.dram_tensor("wq", [n_layers, D, 3 * D], bf16, kind="ExternalInput")
    wp_in = nc.dram_tensor("wp", [n_layers, D, D], bf16, kind="ExternalInput")
    w1_in = nc.dram_tensor("w1", [n_layers, D, D], bf16, kind="ExternalInput")
    w2_in = nc.dram_tensor("w2", [n_layers, D, D], bf16, kind="ExternalInput")
    wo_in = nc.dram_tensor("wo", [D, VS], bf16, kind="ExternalInput")
    mk_in = nc.dram_tensor("masks", [QT, 128, 4 * CHUNK], bf16, kind="ExternalInput")
    out_ext = nc.dram_tensor("logits", [N_CORES * CHUNK, VS], f32, kind="ExternalOutput")

    # per-layer collective buffers (k^T [D,CHUNK] then v [CHUNK,D])
    KVLOC = 2 * D * CHUNK
    kv_locs = [nc.dram_tensor(f"kvl_{l}", [2 * D, CHUNK], bf16) for l in range(n_layers)]
    kv_alls = [nc.dram_tensor(f"kva_{l}", [4 * 2 * D, CHUNK], bf16) for l in range(n_layers)]
    xf_loc = nc.dram_tensor("xfl", [D, CHUNK], bf16)
    xf_all = nc.dram_tensor("xfa", [N_CORES * D, CHUNK], bf16, addr_space="Shared")

    groups_b = [[0, 1, 2, 3], [4, 5, 6, 7]]
    group_all = [list(range(N_CORES))]

    def dram_ap(handle, offset, ap):
        base = handle[:, :]
        return bass.AP(tensor=base.tensor, offset=offset, ap=ap)

    def _patch_tile_name(pool):
        orig = pool.tile
        def tile(shape, dtype, *, tag="", **kw):
            kw.setdefault("name", tag or "t")
            return orig(shape, dtype, tag=tag, **kw)
        pool.tile = tile
        return pool

    with tile.TileContext(nc) as tc, ExitStack() as ctx:
        persist = _patch_tile_name(ctx.enter_context(tc.tile_pool(name="persist", bufs=1)))
        x_t = [persist.tile([128, D], f32, tag=f"x{m}") for m in range(2)]
        for m in range(2):
            nc.sync.dma_start(out=x_t[m], in_=x0_in[128 * m:128 * (m + 1), :])
        ident = persist.tile([128, 128], bf16, tag="ident")
        make_identity(nc, ident)
        eps_t = persist.tile([128, 1], f32, tag="eps")
        nc.vector.memset(eps_t, EPS)
        mask_t = [persist.tile([128, 4 * CHUNK], bf16, tag=f"mask{p}") for p in range(QT)]
        for p in range(QT):
            nc.sync.dma_start(out=mask_t[p], in_=mk_in[p, :, :])
        ones1 = persist.tile([1, 64], bf16, tag="ones1")
        nc.vector.memset(ones1, 1.0)

        ln_pool = _patch_tile_name(ctx.enter_context(tc.tile_pool(name="ln", bufs=1)))
        wqpool = _patch_tile_name(ctx.enter_context(tc.tile_pool(name="wq", bufs=1)))
        wpool = _patch_tile_name(ctx.enter_context(tc.tile_pool(name="wsmall", bufs=1)))
        apool = _patch_tile_name(ctx.enter_context(tc.tile_pool(name="acts", bufs=1)))
        kvpool = _patch_tile_name(ctx.enter_context(tc.tile_pool(name="kv", bufs=1)))
        epool = _patch_tile_name(ctx.enter_context(tc.tile_pool(name="eexp", bufs=1)))
        spool = _patch_tile_name(ctx.enter_context(tc.tile_pool(name="small", bufs=3)))

        ps_q = _patch_tile_name(ctx.enter_context(tc.tile_pool(name="ps_q", bufs=2, space="PSUM")))
        ps_g = _patch_tile_name(ctx.enter_context(tc.tile_pool(name="ps_g", bufs=4, space="PSUM")))

        def layernorm_T(xt, tagpfx):
            """LN both token tiles of x -> feature-major bf16 tiles [128, 256] x8."""
            xh = []
            for m in range(2):
                stats = spool.tile([128, 2, 6], f32, tag="lnstats")
                nc.vector.bn_stats(out=stats[:, 0, :], in_=xt[m][:, 0:512])
                nc.vector.bn_stats(out=stats[:, 1, :], in_=xt[m][:, 512:1024])
                mv = spool.tile([128, 2], f32, tag="lnmv")
                nc.vector.bn_aggr(out=mv, in_=stats)
                rs = spool.tile([128, 1], f32, tag="lnrs")
                nc.scalar.activation(out=rs, in_=mv[:, 1:2], func=AF.Sqrt, bias=eps_t)
                nc.vector.reciprocal(out=rs, in_=rs)
                xh_m = ln_pool.tile([128, D], bf16, tag=f"lnxh{m}")
                nc.vector.tensor_scalar(
                    out=xh_m, in0=xt[m], scalar1=mv[:, 0:1], scalar2=rs,
                    op0=mybir.AluOpType.subtract, op1=mybir.AluOpType.mult)
                xh.append(xh_m)
            xhT = [ln_pool.tile([128, CHUNK], bf16, tag=f"lnxhT{t}") for t in range(FT)]
            for t in range(FT):
                for m in range(2):
                    ptr = ps_g.tile([128, 128], bf16, tag="g", padded_shape=[128, 512])
                    nc.tensor.transpose(ptr, xh[m][:, 128 * t:128 * (t + 1)], ident)
                    eng = nc.vector if (t + m) % 2 == 0 else nc.scalar
                    if eng is nc.scalar:
                        nc.scalar.copy(out=xhT[t][:, 128 * m:128 * (m + 1)], in_=ptr)
                    else:
                        nc.vector.tensor_copy(out=xhT[t][:, 128 * m:128 * (m + 1)], in_=ptr)
            return xhT

        def load_wq(li, what):
            """Emit weight DMAs for layer li's qkv weights. what in {'k','qv'}."""
            for kk in range(FT):
                r0, r1 = 128 * kk, 128 * (kk + 1)
                if what == 'k':
                    nc.sync.dma_start(out=wq_t[kk][:, D:2 * D], in_=wq_in[li, r0:r1, D:2 * D])
                else:
                    nc.sync.dma_start(out=wq_t[kk][:, 0:D], in_=wq_in[li, r0:r1, 0:D])
                    nc.sync.dma_start(out=wq_t[kk][:, 2 * D:3 * D], in_=wq_in[li, r0:r1, 2 * D:3 * D])

        def load_w(dst, src, li):
            for kk in range(FT):
                nc.sync.dma_start(out=dst[kk], in_=src[li, 128 * kk:128 * (kk + 1), :])

        # persistent weight tiles
        wq_t = [wqpool.tile([128, 3 * D], bf16, tag=f"wq{kk}") for kk in range(FT)]
        wp_t = [wpool.tile([128, D], bf16, tag=f"wp{kk}") for kk in range(FT)]
        w1_t = [wpool.tile([128, D], bf16, tag=f"w1{kk}") for kk in range(FT)]
        w2_t = [wpool.tile([128, D], bf16, tag=f"w2{kk}") for kk in range(FT)]

        # prologue: layer 0 weights
        load_wq(0, 'k')
        load_wq(0, 'qv')
        load_w(wp_t, wp_in, 0)
        load_w(w1_t, w1_in, 0)
        load_w(w2_t, w2_in, 0)

        def dense_chain_256(dst_tiles, f_range, col_of, xhT, copy_engines=None):
            """out feature-major tiles: for f in f_range: chain over kk, copy to dst."""
            for i, f in enumerate(f_range):
                ps = ps_g.tile([128, 512], f32, tag="g")
                for kk in range(FT):
                    nc.tensor.matmul(ps[:, 0:CHUNK], wq_t[kk][:, col_of(f):col_of(f) + 128],
                                     xhT[kk], start=(kk == 0), stop=(kk == FT - 1))
                eng = (copy_engines[i % len(copy_engines)] if copy_engines else nc.vector)
                if eng is nc.scalar:
                    nc.scalar.copy(out=dst_tiles[i], in_=ps[:, 0:CHUNK])
                else:
                    eng.tensor_copy(out=dst_tiles[i], in_=ps[:, 0:CHUNK])

        def forward():
            for l in range(n_layers):
                xhT = layernorm_T(x_t, f"ln1_{l}")

                # ---- K and V first, ship, single AllGather ----
                kTl = [apool.tile([128, CHUNK], bf16, tag=f"kT{t}") for t in range(FT)]
                dense_chain_256(kTl, range(FT), lambda f: D + 128 * f, xhT,
                                [nc.vector, nc.scalar])
                kv_loc, kv_all = kv_locs[l], kv_alls[l]
                for t in range(FT):
                    nc.sync.dma_start(
                        out=dram_ap(kv_loc, 128 * t * CHUNK, [[CHUNK, 128], [1, CHUNK]]),
                        in_=kTl[t])
                v_t = [kvpool.tile([128, D], bf16, tag=f"vloc{m}") for m in range(2)]
                for m in range(2):
                    for n in range(2):
                        ps = ps_g.tile([128, 512], f32, tag="g")
                        for kk in range(FT):
                            nc.tensor.matmul(
                                ps, xhT[kk][:, 128 * m:128 * (m + 1)],
                                wq_t[kk][:, 2 * D + 512 * n:2 * D + 512 * (n + 1)],
                                start=(kk == 0), stop=(kk == FT - 1))
                        nc.vector.tensor_copy(out=v_t[m][:, 512 * n:512 * (n + 1)], in_=ps)
                for m in range(2):
                    nc.sync.dma_start(
                        out=dram_ap(kv_loc, D * CHUNK + 128 * m * D, [[D, 128], [1, D]]),
                        in_=v_t[m])
                nc.gpsimd.collective_compute(
                    "AllGather", mybir.AluOpType.bypass, replica_groups=groups_b,
                    ins=[kv_loc[:, :]], outs=[kv_all[:, :]])

                # ---- Q (fills AG window) ----
                qT = [apool.tile([128, CHUNK], bf16, tag=f"qT{t}") for t in range(FT)]
                dense_chain_256(qT, range(FT), lambda f: 128 * f, xhT,
                                [nc.vector, nc.scalar])

                # prefetch next layer's qkv weights (transfers run during attention)
                if l + 1 < n_layers:
                    load_wq(l + 1, 'k')
                    load_wq(l + 1, 'qv')

                # ---- load gathered K^T: tiles [128, 1024] (features x kv tokens) ----
                kall = [kvpool.tile([128, 4 * CHUNK], bf16, tag=f"kall{t}") for t in range(FT)]
                for t in range(FT):
                    nc.sync.dma_start(
                        out=kall[t].rearrange("p (r c) -> p r c", r=4),
                        in_=dram_ap(kv_all, 128 * t * CHUNK,
                                    [[CHUNK, 128], [KVLOC, 4], [1, CHUNK]]))
                # gathered V into 65-strided head-extended layout + ones col
                vext = [kvpool.tile([128, 16 * 65], bf16, tag=f"vext{t}") for t in range(KT)]
                for tt in range(KT):
                    r_, m_ = tt // 2, tt % 2
                    ve = vext[tt].rearrange("p (h c) -> p h c", h=16)
                    nc.sync.dma_start(
                        out=ve[:, :, 0:64],
                        in_=dram_ap(kv_all, KVLOC * r_ + D * CHUNK + 128 * m_ * D,
                                    [[D, 128], [64, 16], [1, 64]]))
                    nc.gpsimd.memset(ve[:, :, 64:65], 1.0)

                # ---- attention: pipelined, av delayed one head ----
                attnT = [apool.tile([128, CHUNK], bf16, tag=f"kT{t}") for t in range(FT)]

                def emit_av(h, em_quads):
                    att_ps = ps_g.tile([65, CHUNK], f32, tag="g")
                    for t in range(KT):
                        nc.tensor.matmul(att_ps, vext[t][:, 65 * h:65 * h + 65],
                                         em_quads[t // 4][:, 256 * (t % 4):256 * (t % 4 + 1)],
                                         start=(t == 0), stop=(t == KT - 1))
                    ft, ro = h // 2, 64 * (h % 2)
                    d_sb = spool.tile([1, CHUNK], f32, tag="denom", bufs=3)
                    nc.vector.tensor_copy(out=d_sb, in_=att_ps[64:65, :])
                    r_sb = spool.tile([1, CHUNK], f32, tag="recip", bufs=3)
                    nc.vector.reciprocal_approx_fast(out=r_sb, in_=d_sb)
                    if bcast_mode == 'dve0':
                        rb = bass.AP(tensor=r_sb[:, :].tensor, offset=r_sb[:, :].offset,
                                     ap=[[0, 64]] + [list(p) for p in r_sb[:, :].ap[1:]])
                        nc.vector.tensor_mul(out=attnT[ft][ro:ro + 64, :],
                                             in0=att_ps[0:64, :], in1=rb)
                    elif bcast_mode == 'pb':
                        rb_sb = spool.tile([64, CHUNK], f32, tag="rbsb", bufs=2)
                        nc.gpsimd.partition_broadcast(rb_sb, r_sb, channels=64)
                        nc.vector.tensor_mul(out=attnT[ft][ro:ro + 64, :],
                                             in0=att_ps[0:64, :], in1=rb_sb)
                    else:  # 'mm'
                        r_bf = spool.tile([1, CHUNK], bf16, tag="recipb", bufs=3)
                        nc.vector.tensor_copy(out=r_bf, in_=r_sb)
                        rb_ps = ps_g.tile([64, CHUNK], f32, tag="g")
                        nc.tensor.matmul(rb_ps, ones1, r_bf, start=True, stop=True)
                        rb_sb = spool.tile([64, CHUNK], f32, tag="rbsb", bufs=2)
                        nc.vector.tensor_copy(out=rb_sb, in_=rb_ps)
                        nc.vector.tensor_mul(out=attnT[ft][ro:ro + 64, :],
                                             in0=att_ps[0:64, :], in1=rb_sb)

                prev = None
                for h in range(H):
                    ft, ro = h // 2, 64 * (h % 2)
                    em_quads = []
                    for p in range(QT):
                        s_ps = ps_q.tile([128, 4 * CHUNK], f32, tag="squad")
                        for tp in range(4):
                            t = 4 * p + tp
                            nc.tensor.matmul(
                                s_ps[:, 256 * tp:256 * (tp + 1)],
                                kall[ft][ro:ro + 64, 128 * t:128 * (t + 1)],
                                qT[ft][ro:ro + 64, :], start=True, stop=True)
                        e_q = epool.tile([128, 4 * CHUNK], bf16, tag="eq", bufs=3)
                        nc.scalar.activation(out=e_q, in_=s_ps, func=AF.Exp, scale=0.125)
                        em_q = epool.tile([128, 4 * CHUNK], bf16, tag="emq", bufs=4)
                        nc.vector.tensor_mul(out=em_q, in0=e_q, in1=mask_t[p])
                        em_quads.append(em_q)
                    if prev is not None:
                        emit_av(*prev)
                    prev = (h, em_quads)
                emit_av(*prev)

                # ---- proj + residual ----
                for m in range(2):
                    for n in range(2):
                        ps = ps_g.tile([128, 512], f32, tag="g")
                        for kk in range(FT):
                            nc.tensor.matmul(
                                ps, attnT[kk][:, 128 * m:128 * (m + 1)],
                                wp_t[kk][:, 512 * n:512 * (n + 1)],
                                start=(kk == 0), stop=(kk == FT - 1))
                        nc.vector.tensor_add(
                            out=x_t[m][:, 512 * n:512 * (n + 1)],
                            in0=x_t[m][:, 512 * n:512 * (n + 1)], in1=ps)
                if l + 1 < n_layers:
                    load_w(wp_t, wp_in, l + 1)

                # ---- MLP ----
                hT = layernorm_T(x_t, f"ln2_{l}")
                gT = [apool.tile([128, CHUNK], bf16, tag=f"qT{t}") for t in range(FT)]
                for f in range(FT):
                    ps = ps_g.tile([128, 512], f32, tag="g")
                    for kk in range(FT):
                        nc.tensor.matmul(ps[:, 0:CHUNK], w1_t[kk][:, 128 * f:128 * (f + 1)],
                                         hT[kk], start=(kk == 0), stop=(kk == FT - 1))
                    nc.scalar.activation(out=gT[f], in_=ps[:, 0:CHUNK], func=AF.Gelu)
                if l + 1 < n_layers:
                    load_w(w1_t, w1_in, l + 1)
                for m in range(2):
                    for n in range(2):
                        ps = ps_g.tile([128, 512], f32, tag="g")
                        for kk in range(FT):
                            nc.tensor.matmul(
                                ps, gT[kk][:, 128 * m:128 * (m + 1)],
                                w2_t[kk][:, 512 * n:512 * (n + 1)],
                                start=(kk == 0), stop=(kk == FT - 1))
                        nc.vector.tensor_add(
                            out=x_t[m][:, 512 * n:512 * (n + 1)],
                            in0=x_t[m][:, 512 * n:512 * (n + 1)], in1=ps)
                if l + 1 < n_layers:
                    load_w(w2_t, w2_in, l + 1)

            # ---- final LN + AllGather (Shared) + logits ----
            xfT = layernorm_T(x_t, "lnf")
            for t in range(FT):
                nc.sync.dma_start(
                    out=dram_ap(xf_loc, 128 * t * CHUNK, [[CHUNK, 128], [1, CHUNK]]),
                    in_=xfT[t])
            nc.gpsimd.collective_compute(
                "AllGather", mybir.AluOpType.bypass, replica_groups=group_all,
                ins=[xf_loc[:, :]], outs=[xf_all[:, :]])
            xall = [wqpool.tile([128, N_CORES * CHUNK], bf16, tag=f"wq{t}")
                    for t in range(FT)]
            for t in range(FT):
                nc.sync.dma_start(
                    out=xall[t].rearrange("p (r c) -> p r c", r=N_CORES),
                    in_=dram_ap(xf_all, 128 * t * CHUNK,
                                [[CHUNK, 128], [D * CHUNK, N_CORES], [1, CHUNK]]))
            NCH = [512] * 7 + [VS - 512 * 7]
            for n in range(8):
                n0 = 512 * n
                won = [wpool.tile([128, 512], bf16, tag=f"won{kk}", bufs=2) for kk in range(FT)]
                for kk in range(FT):
                    nc.sync.dma_start(out=won[kk][:, :NCH[n]],
                                      in_=wo_in[128 * kk:128 * (kk + 1), n0:n0 + NCH[n]])
                for mm in range(16):
                    ps = ps_g.tile([128, 512], f32, tag="g")
                    for kk in range(FT):
                        nc.tensor.matmul(
                            ps[:, :NCH[n]], xall[kk][:, 128 * mm:128 * (mm + 1)],
                            won[kk][:, :NCH[n]],
                            start=(kk == 0), stop=(kk == FT - 1))
                    lg = ln_pool.tile([128, 512], f32, tag=f"lnxh{mm % 2}")
                    eng = nc.vector if mm % 2 == 0 else nc.scalar
                    if eng is nc.scalar:
                        nc.scalar.copy(out=lg[:, :NCH[n]], in_=ps[:, :NCH[n]])
                    else:
                        nc.vector.tensor_copy(out=lg[:, :NCH[n]], in_=ps[:, :NCH[n]])
                    nc.sync.dma_start(
                        out=out_ext[128 * mm:128 * (mm + 1), n0:n0 + NCH[n]],
                        in_=lg[:, :NCH[n]])

        forward()

    nc.compile()
    return nc


_CACHE = {}


def _get_program(n_layers=L, bcast_mode='dve0'):
    key = (n_layers, bcast_mode)
    if key not in _CACHE:
        _CACHE[key] = build_program(n_layers, bcast_mode)
    return _CACHE[key]


def build_in_maps(prep, n_layers=L):
    in_maps = []
    for c in range(N_CORES):
        b, j = c // 4, c % 4
        in_maps.append({
            'x0': np.ascontiguousarray(prep['x0'][b, 256 * j:256 * (j + 1), :]),
            'wq': prep['w_qkv'][:n_layers],
            'wp': prep['w_proj'][:n_layers],
            'w1': prep['w_fc1'][:n_layers],
            'w2': prep['w_fc2'][:n_layers],
            'wo': np.ascontiguousarray(prep['w_out'][:, VS * c:VS * (c + 1)]),
            'masks': make_masks(j),
        })
    return in_maps


def unshard(res, n_rep=1):
    parts = [res.results[c]['logits'] for c in range(N_CORES)]   # [2048, 4000] each
    full = np.concatenate(parts, axis=1)                          # [2048, 32000]
    return full.reshape(B, T, V)


def run_model(prep, n_layers=L, bcast_mode='dve0', **run_kwargs):
    from concourse.bass_utils import run_bass_kernel_spmd
    nc = _get_program(n_layers, bcast_mode)
    in_maps = build_in_maps(prep, n_layers)
    res = run_bass_kernel_spmd(nc, in_maps, core_ids=list(range(N_CORES)), **run_kwargs)
    return unshard(res)


def kernel(**inputs):
    prep = host_prep(inputs)
    return run_model(prep)
